# revision 11
# baseline (speedup 1.0000x reference)
"""GridAttention Trainium2 kernel.

Full inputs -> full output. Internally shards (batch, head-pair) across 8
NeuronCores: core c handles batch c//4 and heads (2*(c%4), 2*(c%4)+1).

Math notes:
 - Attention scores are computed TRANSPOSED: S^T[j, i] = k_j . q_i * scale
   + rowbias[i, j], laid out [k partitions, q free]. This makes softmax-exp
   elementwise, the denominator a matmul reduction (ones column in V), and
   P^T directly usable as the moving operand of the AV matmul.
 - The 2D relative-position bias splits additively:
     bias[i, j] = rowtab[ri-rj+63, h] + coltab[ci-cj+47, h]
   (no clipping needed since H==MAX_H, W==MAX_W).
   * ROW bias rides inside the QK matmul: the contraction is augmented to
     K=128 = [qk 64 | onehot(rj) 64] against [q 64 | rowr 64]; matmul cost
     on TRN2 is N-columns only, so this is free.
   * COL bias is applied MULTIPLICATIVELY after exp: P = exp(qk+row) *
     exp(colbias). exp(colbias)^T tiles are periodic with period 3 in both
     the 128-wide k-chunk index and the 512-wide q-chunk index (since
     128%48=32, 512%48=32, 3*32%48=0), so only 9 distinct [128, 1024]
     pair-blocks exist; host-precomputed, applied as one tensor_mul per
     exp tile (3 of every 12 on the otherwise-idle GpSimd engine, the rest
     on DVE, to keep DVE below the scalar-engine bound).
 - No max-subtraction in softmax: logits ~ N(0,1), exp is safe in fp32/fp16
   and softmax is shift-invariant so results match the reference.
 - Softmax normalization and head combination happen ON HOST: the device
   emits, per head, the UNNORMALIZED projected output (P_h V_h W_h) in fp16
   plus the per-query denominator row; host computes sum_h out_h / d_h.
 - Engine budget per core: scalar exp = 144 tiles x ~1.0us (the ceiling),
   PE = ~150us of N-512 matmuls, DVE ~135us, GpSimd ~80us. PSUM: score
   tiles 2x2 banks, AV accumulators 2x1, output-projection tiles 2x1.
   All side work (q/v projections, output projection) is spread across the
   144 main-loop groups so no engine queue ever bursts.
"""

import numpy as np

EMBED = 512
NH = 8
HD = 64
GH, GW = 64, 48
B = 2
S = GH * GW  # 3072
N_CORES = 8
NQ = S // 512  # 6 q chunks of 512
NM = S // 128  # 24 k chunks of 128
NG = NM // 2   # 12 groups of 2 k-chunks per (n, h)
KC = 4         # 512 = 4 contraction chunks of 128

_CACHE = {}


def _build_program():
    import concourse.bass as bass
    import concourse.tile as tile
    import concourse.mybir as mybir
    from concourse import bacc
    from concourse.bass import ts, ds
    from concourse.masks import make_identity

    f32 = mybir.dt.float32
    f16 = mybir.dt.float16
    EXP = mybir.ActivationFunctionType.Exp

    nc = bacc.Bacc("TRN2", target_bir_lowering=False, debug=False,
                   num_devices=N_CORES)

    def inp(name, shape):
        return nc.dram_tensor(name, shape, f16, kind="ExternalInput").ap()

    # host-prepacked layouts (see _prep_core_inputs)
    xT_d = inp("xT", [128, NQ * 2048])        # [p, n*2048 + c*512 + col]
    wqkv_d = inp("wqkv", [128, 3 * 512])      # [p, (q|k|v)*512 + c*128 + col]
    or_d = [inp(f"or{h}", [64, 2 * S]) for h in range(2)]   # ohr | rowr_h
    ecol_d = inp("ecol", [128, 2 * 9216])     # per head, 9 periodic pairblocks
    wout_d = inp("wout", [HD, 2 * EMBED])
    outa_d = nc.dram_tensor("outa", [S, EMBED], f16, kind="ExternalOutput").ap()
    outb_d = nc.dram_tensor("outb", [S, EMBED], f16, kind="ExternalOutput").ap()
    den_d = nc.dram_tensor("den", [2, S], f16, kind="ExternalOutput").ap()

    with tile.TileContext(nc) as tc:
        with (
            tc.tile_pool(name="const", bufs=1) as cpool,
            tc.tile_pool(name="vtwp", bufs=2) as vtwp,
            tc.tile_pool(name="ptp", bufs=3) as ptp,
            tc.tile_pool(name="ptmp", bufs=5) as ptmp,
            tc.tile_pool(name="osb", bufs=3) as opool,
            tc.tile_pool(name="ps", bufs=2, space="PSUM") as ps,
        ):
            # ---- resident SBUF tensors ----
            xT = cpool.tile([128, NQ * 2048], f16)
            wqkv = cpool.tile([128, 3 * 512], f16)
            wout = cpool.tile([HD, 2 * EMBED], f16)
            # augLR[h]: cols [0:S) = augL (k|onehot-row), [S:2S) = augR (q|rowr)
            augLR = [cpool.tile([128, 2 * S], f16, tag=f"augLR{h}",
                                name=f"augLR{h}") for h in range(2)]
            ecol = cpool.tile([128, 2 * 9216], f16)
            vv = [cpool.tile([128, NM * 65], f16, tag=f"vv{h}", name=f"vv{h}")
                  for h in range(2)]
            outT = [cpool.tile([65, S], f16, tag=f"outT{h}", name=f"outT{h}")
                    for h in range(2)]
            ident = cpool.tile([128, 128], f16)

            # ---- input DMAs (sync queue; ordered by first consumption) ----
            nc.sync.dma_start(out=xT[:, ts(0, 2048)], in_=xT_d[:, ts(0, 2048)])
            nc.sync.dma_start(out=wqkv[:, :], in_=wqkv_d[:, :])
            for n in range(1, NQ):
                nc.sync.dma_start(out=xT[:, ts(n, 2048)],
                                  in_=xT_d[:, ts(n, 2048)])
            nc.sync.dma_start(out=augLR[0][64:128, :], in_=or_d[0][:, :])
            nc.sync.dma_start(out=ecol[:, :], in_=ecol_d[:, :])
            nc.sync.dma_start(out=augLR[1][64:128, :], in_=or_d[1][:, :])
            nc.sync.dma_start(out=wout[:, :], in_=wout_d[:, :])

            make_identity(nc, ident[:, :])
            # ones columns of v_aug (overwritten below for cols 0..63)
            nc.vector.memset(vv[0][:, :], 1.0)
            nc.vector.memset(vv[1][:, :], 1.0)

            def proj(dst_tag, w_ofs, n, tag):
                """4 accumulating matmuls: project x chunk n (M=128, 2 heads)."""
                p = ps.tile([128, 512], f32, tag=tag, name=f"p{dst_tag}")
                for c in range(KC):
                    nc.tensor.matmul(p[:, :], wqkv[:, ds(w_ofs + c * 128, 128)],
                                     xT[:, ds(n * 2048 + c * 512, 512)],
                                     start=(c == 0), stop=(c == KC - 1))
                return p

            def emit_qproj_copy(pq, n, vec):
                for h in range(2):
                    dst = augLR[h][0:64, ds(S + n * 512, 512)]
                    src = pq[64 * h:64 * h + 64, :]
                    if vec:
                        nc.vector.tensor_copy(dst, src)
                    else:
                        nc.scalar.copy(dst, src)

            # ---- main-loop building blocks ----
            groups = [(n, h, g) for n in range(NQ) for h in range(2)
                      for g in range(NG)]
            live = {}
            acc = {}

            def emit_scores(i):
                n, h, g = groups[i]
                st = ps.tile([128, 1024], f32, tag="st", name="st")
                for k in range(2):
                    m = 2 * g + k
                    nc.tensor.matmul(st[:, ts(k, 512)],
                                     augLR[h][:, ts(m, 128)],
                                     augLR[h][:, ds(S + n * 512, 512)],
                                     start=True, stop=True)
                live[("st", i)] = st

            def emit_expmul(i):
                n, h, g = groups[i]
                st = live.pop(("st", i))
                pt = ptp.tile([128, 1024], f16, tag="pt", name="pt")
                nc.scalar.activation(pt[:, :], st[:, :], EXP)
                ptm = ptmp.tile([128, 1024], f16, tag="ptm", name="ptm")
                esl = ecol[:, ds(h * 9216 + (n % 3) * 3072 + (g % 3) * 1024,
                                 1024)]
                nc.vector.tensor_mul(ptm[:, :], pt[:, :], esl)
                live[("ptm", i)] = ptm

            def emit_av(i):
                n, h, g = groups[i]
                ptm = live.pop(("ptm", i))
                if g == 0:
                    acc[(n, h)] = ps.tile([65, 512], f32, tag="acc",
                                          name="acc")
                a = acc[(n, h)]
                for k in range(2):
                    m = 2 * g + k
                    nc.tensor.matmul(a[:, :], vv[h][:, ds(m * 65, 65)],
                                     ptm[:, ts(k, 512)],
                                     start=(m == 0), stop=(m == NM - 1))
                if g == NG - 1:
                    nc.vector.tensor_copy(outT[h][:, ts(n, 512)], a[:, :])
                    del acc[(n, h)]

            def emit_tail_unit(t, h):
                fp = ps.tile([128, 512], f32, tag="fp", name="fp")
                nc.tensor.matmul(fp[:, :], outT[h][0:64, ts(t, 128)],
                                 wout[:, ds(h * EMBED, EMBED)],
                                 start=True, stop=True)
                osb = opool.tile([128, 512], f16, tag="osb", name="osb")
                nc.vector.tensor_copy(osb[:, :], fp[:, :])
                out_d = outa_d if h == 0 else outb_d
                nc.sync.dma_start(out=out_d[ts(t, 128), :], in_=osb[:, :])

            def emit_vproj(n):
                pv = proj("v", 1024, n, "fp")
                vtw = vtwp.tile([128, 512], f16, tag="vtw", name="vtw")
                nc.vector.tensor_copy(vtw[:, :], pv[:, :])
                for mm in range(4):
                    m = n * 4 + mm
                    ptr = ps.tile([128, 128], f16, tag="fp", name="ptr")
                    nc.tensor.transpose(ptr[:, :], vtw[:, ts(mm, 128)],
                                        ident[:, :])
                    nc.vector.tensor_copy(vv[0][:, ds(m * 65, 64)],
                                          ptr[:, 0:64])
                    nc.vector.tensor_copy(vv[1][:, ds(m * 65, 64)],
                                          ptr[:, 64:128])

            # ---- ramp: k proj (pipelined with xT DMAs; evac on then-idle
            # DVE), q0; v projections trickle in during the first groups ----
            for n in range(NQ):
                pk = proj("k", 512, n, "st")
                for h in range(2):
                    nc.vector.tensor_copy(augLR[h][0:64, ts(n, 512)],
                                          pk[64 * h:64 * h + 64, :])
            pq0 = proj("q", 0, 0, "st")
            emit_qproj_copy(pq0, 0, vec=True)

            emit_scores(0)
            emit_expmul(0)

            # ---- main loop (AV lagged 3 groups behind scores/exp) ----
            pq = {}
            NGRP = len(groups)
            for i in range(1, NGRP):
                emit_scores(i)
                emit_expmul(i)
                if i >= 3:
                    emit_av(i - 3)   # before side work: emits the outT copy
                # v-proj chunk j lands just before av(2j) consumes it
                if i % 2 == 1 and i <= 11:
                    emit_vproj(i // 2)
                n, h, g = groups[i]
                if i == 15:
                    pq[1] = proj("q", 0, 1, "fp")
                if i == 17:
                    emit_qproj_copy(pq.pop(1), 1, vec=True)
                if 2 <= g < 6:
                    if h == 0 and n >= 1:
                        emit_tail_unit(4 * (n - 1) + (g - 2), 1)
                    elif h == 1:
                        emit_tail_unit(4 * n + (g - 2), 0)
                if h == 0 and g == 6 and 2 <= n + 2 < NQ:
                    pq[n + 2] = proj("q", 0, n + 2, "fp")
                if h == 0 and g == 8 and 2 <= n + 2 < NQ:
                    emit_qproj_copy(pq.pop(n + 2), n + 2, vec=True)
            for i in range(NGRP - 3, NGRP):
                emit_av(i)
            for tt in range(4):
                emit_tail_unit(4 * (NQ - 1) + tt, 1)

            # denominator rows (row 64 of outT = sum_k P)
            nc.sync.dma_start(out=den_d[0:1, :], in_=outT[0][64:65, :])
            nc.sync.dma_start(out=den_d[1:2, :], in_=outT[1][64:65, :])

    nc.compile()
    return nc


def _get_nc():
    if "nc" not in _CACHE:
        _CACHE["nc"] = _build_program()
    return _CACHE["nc"]


def _prep_core_inputs(x, w_qkv, w_out, rel_row_tab, rel_col_tab):
    """Per-core input dicts (host-side shard + constant precompute)."""
    bf = np.float16
    x = np.asarray(x, np.float32)
    w_qkv = np.asarray(w_qkv, np.float32)
    w_out = np.asarray(w_out, np.float32)
    rel_row_tab = np.asarray(rel_row_tab, np.float32)
    rel_col_tab = np.asarray(rel_col_tab, np.float32)

    ri = np.arange(S) // GW           # grid row of flat index
    ci = np.arange(S) % GW            # grid col of flat index
    ohr = (ri[None, :] == np.arange(64)[:, None]).astype(np.float32)
    # rowr[h][t, i] = rel_row_tab[ri[i] - t + 63, h]; idx in [0,126] (no clip)
    row_idx = ri[None, :] - np.arange(64)[:, None] + 63   # [64, S]

    # ecol pair-blocks: for group g (k-chunks m=2g, 2g+1) and q chunk n, the
    # exp(colbias^T) tile depends only on (g%3, n%3): 9 blocks of [128, 1024].
    # [jj, ii'] of block (q3, g3): k-chunk m = 2*g3 + (ii' // 512),
    # coltab[(32*q3 + ii'%512) % 48 - (32*m + jj) % 48 + 47]
    jj = np.arange(128)
    ii = np.arange(512)
    ecol_idx = np.zeros((3, 3, 128, 1024), np.int64)
    for q3 in range(3):
        for g3 in range(3):
            for k in range(2):
                m = (2 * g3 + k) % 3
                cj = (32 * m + jj) % 48
                c_i = (32 * q3 + ii) % 48
                ecol_idx[q3, g3, :, k * 512:(k + 1) * 512] = \
                    c_i[None, :] - cj[:, None] + 47
    # layout: [128, q3 * 3072 + g3 * 1024 + ii']
    ecol_idx = ecol_idx.transpose(2, 0, 1, 3).reshape(128, 9216)

    scale = HD ** -0.5
    in_maps = []
    for c in range(N_CORES):
        b = c // 4
        h0 = 2 * (c % 4)
        h1 = h0 + 1
        xT = np.ascontiguousarray(x[b].reshape(S, EMBED).T)   # [E, S]
        # device layout: [p, n*2048 + c*512 + col] = xT[c*128+p, n*512+col]
        xTn = xT.reshape(KC, 128, NQ, 512).transpose(1, 2, 0, 3)
        def wslice(base, h):
            return w_qkv[:, base + h * HD: base + (h + 1) * HD]
        def pack(base, mul=1.0):
            w = np.concatenate([wslice(base, h0), wslice(base, h1)],
                               axis=1) * mul                  # [512, 128]
            return w.reshape(KC, 128, 128).transpose(1, 0, 2).reshape(128, 512)
        wqkv = np.concatenate([pack(0, scale), pack(EMBED), pack(2 * EMBED)],
                              axis=1)
        ecol_blocks = np.concatenate(
            [np.exp(rel_col_tab[ecol_idx, h0]),
             np.exp(rel_col_tab[ecol_idx, h1])], axis=1)
        in_maps.append({
            "xT": np.ascontiguousarray(xTn.reshape(128, NQ * 2048)).astype(bf),
            "wqkv": np.ascontiguousarray(wqkv).astype(bf),
            "or0": np.concatenate(
                [ohr, rel_row_tab[row_idx, h0]], axis=1).astype(bf),
            "or1": np.concatenate(
                [ohr, rel_row_tab[row_idx, h1]], axis=1).astype(bf),
            "ecol": np.ascontiguousarray(ecol_blocks).astype(bf),
            "wout": np.concatenate(
                [w_out[h0 * HD:(h0 + 1) * HD, :],
                 w_out[h1 * HD:(h1 + 1) * HD, :]], axis=1).astype(bf),
        })
    return in_maps


def _run(inputs, trace=False):
    from concourse.bass_utils import run_bass_kernel_spmd
    nc = _get_nc()
    in_maps = _prep_core_inputs(**inputs)
    res = run_bass_kernel_spmd(nc, in_maps, list(range(N_CORES)), trace=trace)
    acc = np.zeros((B, S, EMBED), np.float32)
    for c in range(N_CORES):
        r = res.results[c]
        den = np.asarray(r["den"], np.float32)          # [2, S]
        acc[c // 4] += np.asarray(r["outa"], np.float32) / den[0][:, None]
        acc[c // 4] += np.asarray(r["outb"], np.float32) / den[1][:, None]
    return acc.reshape(B, GH, GW, EMBED), res


def kernel(x, w_qkv, w_out, rel_row_tab, rel_col_tab):
    out, _ = _run(dict(x=x, w_qkv=w_qkv, w_out=w_out,
                       rel_row_tab=rel_row_tab, rel_col_tab=rel_col_tab))
    return out


# revision 13
# speedup vs baseline: 1.1882x; 1.1882x over previous
"""GridAttention Trainium2 kernel.

Full inputs -> full output. Internally shards (batch, head-pair) across 8
NeuronCores: core c handles batch c//4 and heads (2*(c%4), 2*(c%4)+1).

Math notes:
 - Attention scores are computed TRANSPOSED: S^T[j, i] = k_j . q_i * scale
   + rowbias[i, j], laid out [k partitions, q free]. This makes softmax-exp
   elementwise, the denominator a matmul reduction (ones column in V), and
   P^T directly usable as the moving operand of the AV matmul.
 - The 2D relative-position bias splits additively:
     bias[i, j] = rowtab[ri-rj+63, h] + coltab[ci-cj+47, h]
   (no clipping needed since H==MAX_H, W==MAX_W).
   * ROW bias rides inside the QK matmul: the contraction is augmented to
     K=128 = [qk 64 | onehot(rj) 64] against [q 64 | rowr 64]; matmul cost
     on TRN2 is N-columns only, so this is free.
   * COL bias is applied MULTIPLICATIVELY after exp: P = exp(qk+row) *
     exp(colbias). exp(colbias)^T tiles are periodic with period 3 in both
     the 128-wide k-chunk index and the 512-wide q-chunk index (since
     128%48=32, 512%48=32, 3*32%48=0), so only 9 distinct [128, 1024]
     pair-blocks exist; host-precomputed, applied as one tensor_mul per
     exp tile (3 of every 12 on the otherwise-idle GpSimd engine, the rest
     on DVE, to keep DVE below the scalar-engine bound).
 - No max-subtraction in softmax: logits ~ N(0,1), exp is safe in fp32/fp16
   and softmax is shift-invariant so results match the reference.
 - Softmax normalization and head combination happen ON HOST: the device
   emits, per head, the UNNORMALIZED projected output (P_h V_h W_h) in fp16
   plus the per-query denominator row; host computes sum_h out_h / d_h.
 - Engine budget per core: scalar exp = 144 tiles x ~1.0us (the ceiling),
   PE = ~150us of N-512 matmuls, DVE ~135us, GpSimd ~80us. PSUM: score
   tiles 2x2 banks, AV accumulators 2x1, output-projection tiles 2x1.
   All side work (q/v projections, output projection) is spread across the
   144 main-loop groups so no engine queue ever bursts.
"""

import numpy as np

EMBED = 512
NH = 8
HD = 64
GH, GW = 64, 48
B = 2
S = GH * GW  # 3072
N_CORES = 8
NQ = S // 512  # 6 q chunks of 512
NM = S // 128  # 24 k chunks of 128
NG = NM // 2   # 12 groups of 2 k-chunks per (n, h)
KC = 4         # 512 = 4 contraction chunks of 128

_CACHE = {}


def _build_program():
    import concourse.bass as bass
    import concourse.tile as tile
    import concourse.mybir as mybir
    from concourse import bacc
    from concourse.bass import ts, ds
    from concourse.masks import make_identity

    f32 = mybir.dt.float32
    f16 = mybir.dt.float16
    EXP = mybir.ActivationFunctionType.Exp

    nc = bacc.Bacc("TRN2", target_bir_lowering=False, debug=False,
                   num_devices=N_CORES)

    def inp(name, shape):
        return nc.dram_tensor(name, shape, f16, kind="ExternalInput").ap()

    # host-prepacked layouts (see _prep_core_inputs)
    xT_d = inp("xT", [128, NQ * 2048])        # [p, n*2048 + c*512 + col]
    wqkv_d = inp("wqkv", [128, 3 * 512])      # [p, (q|k|v)*512 + c*128 + col]
    or_d = [inp(f"or{h}", [64, 2 * S]) for h in range(2)]   # ohr | rowr_h
    ecol_d = inp("ecol", [128, 2 * 9216])     # per head, 9 periodic pairblocks
    wout_d = inp("wout", [HD, 2 * EMBED])
    outa_d = nc.dram_tensor("outa", [S, EMBED], f16, kind="ExternalOutput").ap()
    outb_d = nc.dram_tensor("outb", [S, EMBED], f16, kind="ExternalOutput").ap()
    den_d = nc.dram_tensor("den", [2, S], f16, kind="ExternalOutput").ap()

    with tile.TileContext(nc) as tc:
        with (
            tc.tile_pool(name="const", bufs=1) as cpool,
            tc.tile_pool(name="vtwp", bufs=2) as vtwp,
            tc.tile_pool(name="ptp", bufs=3) as ptp,
            tc.tile_pool(name="ptmp", bufs=5) as ptmp,
            tc.tile_pool(name="osb", bufs=3) as opool,
            tc.tile_pool(name="ps", bufs=2, space="PSUM") as ps,
        ):
            # ---- resident SBUF tensors ----
            xT = cpool.tile([128, NQ * 2048], f16)
            wqkv = cpool.tile([128, 3 * 512], f16)
            wout = cpool.tile([HD, 2 * EMBED], f16)
            # augLR[h]: cols [0:S) = augL (k|onehot-row), [S:2S) = augR (q|rowr)
            augLR = [cpool.tile([128, 2 * S], f16, tag=f"augLR{h}",
                                name=f"augLR{h}") for h in range(2)]
            ecol = cpool.tile([128, 2 * 9216], f16)
            vv = [cpool.tile([128, NM * 65], f16, tag=f"vv{h}", name=f"vv{h}")
                  for h in range(2)]
            outT = [cpool.tile([65, S], f16, tag=f"outT{h}", name=f"outT{h}")
                    for h in range(2)]
            ident = cpool.tile([128, 128], f16)

            # ---- input DMAs (sync queue; ordered by first consumption) ----
            nc.sync.dma_start(out=xT[:, ts(0, 2048)], in_=xT_d[:, ts(0, 2048)])
            nc.sync.dma_start(out=wqkv[:, :], in_=wqkv_d[:, :])
            for n in range(1, NQ):
                nc.sync.dma_start(out=xT[:, ts(n, 2048)],
                                  in_=xT_d[:, ts(n, 2048)])
            nc.sync.dma_start(out=augLR[0][64:128, :], in_=or_d[0][:, :])
            nc.sync.dma_start(out=ecol[:, :], in_=ecol_d[:, :])
            nc.sync.dma_start(out=augLR[1][64:128, :], in_=or_d[1][:, :])
            nc.sync.dma_start(out=wout[:, :], in_=wout_d[:, :])

            make_identity(nc, ident[:, :])
            # ones columns of v_aug (overwritten below for cols 0..63)
            nc.vector.memset(vv[0][:, :], 1.0)
            nc.vector.memset(vv[1][:, :], 1.0)

            def proj(dst_tag, w_ofs, n, tag):
                """4 accumulating matmuls: project x chunk n (M=128, 2 heads)."""
                p = ps.tile([128, 512], f32, tag=tag, name=f"p{dst_tag}")
                for c in range(KC):
                    nc.tensor.matmul(p[:, :], wqkv[:, ds(w_ofs + c * 128, 128)],
                                     xT[:, ds(n * 2048 + c * 512, 512)],
                                     start=(c == 0), stop=(c == KC - 1))
                return p

            def emit_qproj_copy(pq, n, vec):
                for h in range(2):
                    dst = augLR[h][0:64, ds(S + n * 512, 512)]
                    src = pq[64 * h:64 * h + 64, :]
                    if vec:
                        nc.vector.tensor_copy(dst, src)
                    else:
                        nc.scalar.copy(dst, src)

            # ---- main-loop building blocks ----
            groups = [(n, h, g) for n in range(NQ) for h in range(2)
                      for g in range(NG)]
            live = {}
            acc = {}

            def emit_scores(i):
                n, h, g = groups[i]
                st = ps.tile([128, 1024], f32, tag="st", name="st")
                for k in range(2):
                    m = 2 * g + k
                    nc.tensor.matmul(st[:, ts(k, 512)],
                                     augLR[h][:, ts(m, 128)],
                                     augLR[h][:, ds(S + n * 512, 512)],
                                     start=True, stop=True)
                live[("st", i)] = st

            def emit_expmul(i):
                n, h, g = groups[i]
                st = live.pop(("st", i))
                pt = ptp.tile([128, 1024], f16, tag="pt", name="pt")
                nc.scalar.activation(pt[:, :], st[:, :], EXP)
                ptm = ptmp.tile([128, 1024], f16, tag="ptm", name="ptm")
                esl = ecol[:, ds(h * 9216 + (n % 3) * 3072 + (g % 3) * 1024,
                                 1024)]
                nc.vector.tensor_mul(ptm[:, :], pt[:, :], esl)
                live[("ptm", i)] = ptm

            def emit_av(i):
                n, h, g = groups[i]
                ptm = live.pop(("ptm", i))
                if g == 0:
                    acc[(n, h)] = ps.tile([65, 512], f32, tag="acc",
                                          name="acc")
                a = acc[(n, h)]
                for k in range(2):
                    m = 2 * g + k
                    nc.tensor.matmul(a[:, :], vv[h][:, ds(m * 65, 65)],
                                     ptm[:, ts(k, 512)],
                                     start=(m == 0), stop=(m == NM - 1))
                if g == NG - 1:
                    nc.vector.tensor_copy(outT[h][:, ts(n, 512)], a[:, :])
                    del acc[(n, h)]

            def emit_tail_unit(t, h):
                fp = ps.tile([128, 512], f32, tag="fp", name="fp")
                nc.tensor.matmul(fp[:, :], outT[h][0:64, ts(t, 128)],
                                 wout[:, ds(h * EMBED, EMBED)],
                                 start=True, stop=True)
                osb = opool.tile([128, 512], f16, tag="osb", name="osb")
                nc.vector.tensor_copy(osb[:, :], fp[:, :])
                out_d = outa_d if h == 0 else outb_d
                nc.sync.dma_start(out=out_d[ts(t, 128), :], in_=osb[:, :])

            def emit_vproj(n):
                pv = proj("v", 1024, n, "fp")
                vtw = vtwp.tile([128, 512], f16, tag="vtw", name="vtw")
                nc.scalar.copy(vtw[:, :], pv[:, :])
                for mm in range(4):
                    m = n * 4 + mm
                    ptr = ps.tile([128, 128], f16, tag="fp", name="ptr")
                    nc.tensor.transpose(ptr[:, :], vtw[:, ts(mm, 128)],
                                        ident[:, :])
                    nc.vector.tensor_copy(vv[0][:, ds(m * 65, 64)],
                                          ptr[:, 0:64])
                    nc.vector.tensor_copy(vv[1][:, ds(m * 65, 64)],
                                          ptr[:, 64:128])

            # ---- ramp: k proj (pipelined with xT DMAs; evac on then-idle
            # DVE), q0; v projections trickle in during the first groups ----
            for n in range(NQ):
                pk = proj("k", 512, n, "st")
                # h0 on scalar (gates the first scores), h1 on DVE (12
                # groups of slack before (0, 1) needs it)
                nc.scalar.copy(augLR[0][0:64, ts(n, 512)], pk[0:64, :])
                nc.vector.tensor_copy(augLR[1][0:64, ts(n, 512)],
                                      pk[64:128, :])
            pq0 = proj("q", 0, 0, "st")
            nc.scalar.copy(augLR[0][0:64, ds(S, 512)], pq0[0:64, :])
            nc.vector.tensor_copy(augLR[1][0:64, ds(S, 512)], pq0[64:128, :])

            emit_scores(0)
            emit_expmul(0)

            # ---- main loop (AV lagged 3 groups behind scores/exp) ----
            pq = {}
            NGRP = len(groups)
            for i in range(1, NGRP):
                emit_scores(i)
                emit_expmul(i)
                if i >= 3:
                    emit_av(i - 3)   # before side work: emits the outT copy
                # v-proj chunk j lands just before av(2j) consumes it
                if i % 2 == 1 and i <= 11:
                    emit_vproj(i // 2)
                n, h, g = groups[i]
                if i == 15:
                    pq[1] = proj("q", 0, 1, "fp")
                if i == 17:
                    emit_qproj_copy(pq.pop(1), 1, vec=True)
                if 2 <= g < 6:
                    if h == 0 and n >= 1:
                        emit_tail_unit(4 * (n - 1) + (g - 2), 1)
                    elif h == 1:
                        emit_tail_unit(4 * n + (g - 2), 0)
                if h == 0 and g == 6 and 2 <= n + 2 < NQ:
                    pq[n + 2] = proj("q", 0, n + 2, "fp")
                if h == 0 and g == 8 and 2 <= n + 2 < NQ:
                    emit_qproj_copy(pq.pop(n + 2), n + 2, vec=True)
            for i in range(NGRP - 3, NGRP):
                emit_av(i)
            for tt in range(4):
                emit_tail_unit(4 * (NQ - 1) + tt, 1)

            # denominator rows (row 64 of outT = sum_k P)
            nc.sync.dma_start(out=den_d[0:1, :], in_=outT[0][64:65, :])
            nc.sync.dma_start(out=den_d[1:2, :], in_=outT[1][64:65, :])

    nc.compile()
    return nc


def _get_nc():
    if "nc" not in _CACHE:
        _CACHE["nc"] = _build_program()
    return _CACHE["nc"]


def _prep_core_inputs(x, w_qkv, w_out, rel_row_tab, rel_col_tab):
    """Per-core input dicts (host-side shard + constant precompute)."""
    bf = np.float16
    x = np.asarray(x, np.float32)
    w_qkv = np.asarray(w_qkv, np.float32)
    w_out = np.asarray(w_out, np.float32)
    rel_row_tab = np.asarray(rel_row_tab, np.float32)
    rel_col_tab = np.asarray(rel_col_tab, np.float32)

    ri = np.arange(S) // GW           # grid row of flat index
    ci = np.arange(S) % GW            # grid col of flat index
    ohr = (ri[None, :] == np.arange(64)[:, None]).astype(np.float32)
    # rowr[h][t, i] = rel_row_tab[ri[i] - t + 63, h]; idx in [0,126] (no clip)
    row_idx = ri[None, :] - np.arange(64)[:, None] + 63   # [64, S]

    # ecol pair-blocks: for group g (k-chunks m=2g, 2g+1) and q chunk n, the
    # exp(colbias^T) tile depends only on (g%3, n%3): 9 blocks of [128, 1024].
    # [jj, ii'] of block (q3, g3): k-chunk m = 2*g3 + (ii' // 512),
    # coltab[(32*q3 + ii'%512) % 48 - (32*m + jj) % 48 + 47]
    jj = np.arange(128)
    ii = np.arange(512)
    ecol_idx = np.zeros((3, 3, 128, 1024), np.int64)
    for q3 in range(3):
        for g3 in range(3):
            for k in range(2):
                m = (2 * g3 + k) % 3
                cj = (32 * m + jj) % 48
                c_i = (32 * q3 + ii) % 48
                ecol_idx[q3, g3, :, k * 512:(k + 1) * 512] = \
                    c_i[None, :] - cj[:, None] + 47
    # layout: [128, q3 * 3072 + g3 * 1024 + ii']
    ecol_idx = ecol_idx.transpose(2, 0, 1, 3).reshape(128, 9216)

    scale = HD ** -0.5
    in_maps = []
    for c in range(N_CORES):
        b = c // 4
        h0 = 2 * (c % 4)
        h1 = h0 + 1
        xT = np.ascontiguousarray(x[b].reshape(S, EMBED).T)   # [E, S]
        # device layout: [p, n*2048 + c*512 + col] = xT[c*128+p, n*512+col]
        xTn = xT.reshape(KC, 128, NQ, 512).transpose(1, 2, 0, 3)
        def wslice(base, h):
            return w_qkv[:, base + h * HD: base + (h + 1) * HD]
        def pack(base, mul=1.0):
            w = np.concatenate([wslice(base, h0), wslice(base, h1)],
                               axis=1) * mul                  # [512, 128]
            return w.reshape(KC, 128, 128).transpose(1, 0, 2).reshape(128, 512)
        wqkv = np.concatenate([pack(0, scale), pack(EMBED), pack(2 * EMBED)],
                              axis=1)
        ecol_blocks = np.concatenate(
            [np.exp(rel_col_tab[ecol_idx, h0]),
             np.exp(rel_col_tab[ecol_idx, h1])], axis=1)
        in_maps.append({
            "xT": np.ascontiguousarray(xTn.reshape(128, NQ * 2048)).astype(bf),
            "wqkv": np.ascontiguousarray(wqkv).astype(bf),
            "or0": np.concatenate(
                [ohr, rel_row_tab[row_idx, h0]], axis=1).astype(bf),
            "or1": np.concatenate(
                [ohr, rel_row_tab[row_idx, h1]], axis=1).astype(bf),
            "ecol": np.ascontiguousarray(ecol_blocks).astype(bf),
            "wout": np.concatenate(
                [w_out[h0 * HD:(h0 + 1) * HD, :],
                 w_out[h1 * HD:(h1 + 1) * HD, :]], axis=1).astype(bf),
        })
    return in_maps


def _run(inputs, trace=False):
    from concourse.bass_utils import run_bass_kernel_spmd
    nc = _get_nc()
    in_maps = _prep_core_inputs(**inputs)
    res = run_bass_kernel_spmd(nc, in_maps, list(range(N_CORES)), trace=trace)
    acc = np.zeros((B, S, EMBED), np.float32)
    for c in range(N_CORES):
        r = res.results[c]
        den = np.asarray(r["den"], np.float32)          # [2, S]
        acc[c // 4] += np.asarray(r["outa"], np.float32) / den[0][:, None]
        acc[c // 4] += np.asarray(r["outb"], np.float32) / den[1][:, None]
    return acc.reshape(B, GH, GW, EMBED), res


def kernel(x, w_qkv, w_out, rel_row_tab, rel_col_tab):
    out, _ = _run(dict(x=x, w_qkv=w_qkv, w_out=w_out,
                       rel_row_tab=rel_row_tab, rel_col_tab=rel_col_tab))
    return out
